# revision 6
# baseline (speedup 1.0000x reference)
"""DeepSpeed transformer-inference block on 8 TRN2 NeuronCores.

Sharding (tensor-parallel attention + row-parallel MLP):
  - LN1 is row-parallel: each core normalizes its own 512 rows, transposes
    them to feature-major bf16, and an AllGather distributes t1^T to all cores.
  - QKV GEMM is head-parallel: each core computes Q/K/V for its 2 heads over
    all 4096 rows (weight columns sharded host-side).
  - Attention (causal, no-max-sub softmax) runs per-core on its 2 heads in
    transposed layout; the softmax denominator comes from a ones-column
    appended to V.  ctx^T (normalized, bf16) is exchanged with an AllToAll so
    every core ends up with the full 1024 ctx features for its own 512 rows.
  - attn output projection, residual, LN2 and the whole MLP are row-parallel
    with full (bf16) weights; final 512x1024 fp32 row chunk per core is
    gathered host-side.

LN gains/biases are folded into the following GEMM's weights/bias host-side,
so the device only computes (x - mu) * rstd.
"""

import numpy as np
import ml_dtypes

import concourse.bass as bass
import concourse.mybir as mybir
import concourse.tile as tile
from concourse import bacc
from concourse.bass_utils import run_bass_kernel_spmd

P = 128
B, S, H, I = 2, 2048, 1024, 4096
NH, HD = 16, 64
NC = 8                      # cores
HPC = NH // NC              # heads per core (2)
RPC = B * S // NC           # rows per core (512)
NROWS = B * S               # 4096
QB = 512                    # query block (free dim) in attention
NKT = S // P                # 16 k-tiles per batch
SCALE = 1.0 / np.sqrt(HD)   # 0.125
EPS = 1e-12
BF = mybir.dt.bfloat16
F32 = mybir.dt.float32
RG = [list(range(NC))]

AF = mybir.ActivationFunctionType

# Info about the last hardware run (exec time etc), for test harnesses.
last_run_info = {}

_CACHED = None


def _build():
    nc = bacc.Bacc(trn_type="TRN2", target_bir_lowering=False,
                   num_devices=NC)

    # ---- I/O -------------------------------------------------------------
    x_rows = nc.dram_tensor("x_rows", [RPC, H], F32, kind="ExternalInput")
    qkvw_s = nc.dram_tensor("qkvw_s", [H, 3 * P], BF, kind="ExternalInput")
    qkvb_s = nc.dram_tensor("qkvb_s", [P, 3], F32, kind="ExternalInput")
    attn_ow = nc.dram_tensor("attn_ow", [H, H], BF, kind="ExternalInput")
    attn_ob = nc.dram_tensor("attn_ob", [1, H], BF, kind="ExternalInput")
    interw = nc.dram_tensor("interw", [H, I], BF, kind="ExternalInput")
    interb = nc.dram_tensor("interb", [P, I // P], F32, kind="ExternalInput")
    outw = nc.dram_tensor("outw", [I, H], BF, kind="ExternalInput")
    outb = nc.dram_tensor("outb", [1, H], BF, kind="ExternalInput")
    masks_in = nc.dram_tensor("masks", [P, 4, QB], BF, kind="ExternalInput")
    out = nc.dram_tensor("out", [RPC, H], F32, kind="ExternalOutput")

    with tile.TileContext(nc) as tc:
        with tc.tile_pool(name="ccb", bufs=1, space="DRAM") as ccp, \
             tc.tile_pool(name="persist", bufs=1) as pers:
            _body(nc, tc, ccp, pers, x_rows, qkvw_s, qkvb_s, attn_ow, attn_ob,
                  interw, interb, outw, outb, masks_in, out)
    nc.finalize()
    return nc


def _body(nc, tc, ccp, pers, x_rows, qkvw_s, qkvb_s, attn_ow, attn_ob,
          interw, interb, outw, outb, masks_in, out):
    # collective buffers (internal DRAM; outputs in shared scratchpad)
    cc_t_in = ccp.tile([H, RPC], BF, name="cc_t_in", tag="cc_t_in")
    cc_t_out = ccp.tile([NC * H, RPC], BF, name="cc_t_out", tag="cc_t_out",
                        addr_space="Shared")
    cc_c_in = ccp.tile([NC * P, RPC], BF, name="cc_c_in", tag="cc_c_in")
    cc_c_out = ccp.tile([NC * P, RPC], BF, name="cc_c_out", tag="cc_c_out")

    # ---- persistent SBUF -------------------------------------------------
    x_all = pers.tile([P, 4, H], F32, name="x_all", tag="x_all")
    qkvT = [pers.tile([P, NROWS], BF, name=f"qkvT{i}", tag=f"qkvT{i}")
            for i in range(3)]
    ctxT_all = pers.tile([P, NC, RPC], BF, name="ctxT_all", tag="ctxT_all")
    resid_all = pers.tile([P, 4, H], F32, name="resid_all", tag="resid_all")
    t2T_all = pers.tile([P, 8, RPC], BF, name="t2T_all", tag="t2T_all")
    interT = pers.tile([P, I // P, RPC], BF, name="interT", tag="interT")
    masks_sb = pers.tile([P, 4, QB], BF, name="masks_sb", tag="masks_sb")
    aow_sb = pers.tile([P, 8, H], BF, name="aow_sb", tag="aow_sb")
    qkvw_sb = pers.tile([P, 8, 3 * P], BF, name="qkvw_sb", tag="qkvw_sb")
    qkvb_sb = pers.tile([P, 3], F32, name="qkvb_sb", tag="qkvb_sb")
    interb_sb = pers.tile([P, I // P], F32, name="interb_sb", tag="interb_sb")
    aob_sb = pers.tile([1, H], BF, name="aob_sb", tag="aob_sb")
    outb_sb = pers.tile([1, H], BF, name="outb_sb", tag="outb_sb")
    ones_sb = pers.tile([1, P], BF, name="ones_sb", tag="ones_sb")
    eps_sb = pers.tile([P, 1], F32, name="eps_sb", tag="eps_sb")

    nc.vector.memset(ones_sb[:], 1.0)
    nc.vector.memset(eps_sb[:], EPS)
    nc.sync.dma_start(out=masks_sb[:], in_=masks_in[:])
    nc.sync.dma_start(out=qkvb_sb[:], in_=qkvb_s[:])
    nc.sync.dma_start(out=interb_sb[:], in_=interb[:])
    nc.sync.dma_start(out=aob_sb[:], in_=attn_ob[:])
    nc.sync.dma_start(out=outb_sb[:], in_=outb[:])
    aow_r = attn_ow[:].rearrange("(c p) h -> c p h", p=P)
    qkvw_r = qkvw_s[:].rearrange("(c p) h -> c p h", p=P)
    for hc in range(8):
        nc.sync.dma_start(out=aow_sb[:, hc, :], in_=aow_r[hc])
        nc.sync.dma_start(out=qkvw_sb[:, hc, :], in_=qkvw_r[hc])

    x_r = x_rows[:].rearrange("(t p) h -> t p h", p=P)

    # ============ Stage A: LN1 on own rows, transpose, AllGather ==========
    with tc.tile_pool(name="ln_pool", bufs=3) as lnp, \
         tc.tile_pool(name="tT_pool", bufs=4) as tTp:
        for rt in range(4):
            nc.sync.dma_start(out=x_all[:, rt, :], in_=x_r[rt])
            stats = lnp.tile([P, 2, 6], F32, tag="stats")
            mv = lnp.tile([P, 2], F32, tag="mv")
            nc.vector.bn_stats(out=stats[:, 0, :], in_=x_all[:, rt, 0:512])
            nc.vector.bn_stats(out=stats[:, 1, :], in_=x_all[:, rt, 512:H])
            nc.vector.bn_aggr(out=mv[:], in_=stats[:])
            std = lnp.tile([P, 1], F32, tag="std")
            nc.scalar.activation(out=std[:], in_=mv[:, 1:2], func=AF.Sqrt,
                                 bias=eps_sb[:], scale=1.0)
            rstd = lnp.tile([P, 1], F32, tag="rstd")
            nc.vector.reciprocal(out=rstd[:], in_=std[:])
            nmr = lnp.tile([P, 1], F32, tag="nmr")
            nc.vector.tensor_mul(nmr[:], mv[:, 0:1], rstd[:])
            nc.vector.tensor_scalar_mul(nmr[:], nmr[:], -1.0)
            t_bf = lnp.tile([P, H], BF, tag="t_bf")
            nc.scalar.activation(out=t_bf[:], in_=x_all[:, rt, :],
                                 func=AF.Identity, bias=nmr[:], scale=rstd[:])
            for fc in range(8):
                tt = tTp.tile([P, P], BF, tag="tt")
                nc.sync.dma_start(out=tt[:], in_=t_bf[:, fc * P:(fc + 1) * P],
                                  transpose=True)
                nc.sync.dma_start(
                    out=cc_t_in[fc * P:(fc + 1) * P, rt * P:(rt + 1) * P],
                    in_=tt[:])

    nc.gpsimd.collective_compute(
        "AllGather", mybir.AluOpType.bypass, replica_groups=RG,
        ins=[cc_t_in[:]], outs=[cc_t_out[:]])

    # ============ Stage B: QKV^T over all rows ============================
    with tc.tile_pool(name="qkv_rhs", bufs=10) as rhp, \
         tc.tile_pool(name="qkv_ps", bufs=3, space="PSUM") as qps:
        for blk in range(8):
            rhs = []
            for hc in range(8):
                r = rhp.tile([P, QB], BF, tag="rhs")
                nc.sync.dma_start(
                    out=r[:],
                    in_=cc_t_out[blk * H + hc * P: blk * H + (hc + 1) * P, :])
                rhs.append(r)
            for oc in range(3):
                ps = qps.tile([P, QB], F32, tag="qkvps")
                for hc in range(8):
                    nc.tensor.matmul(ps[:], qkvw_sb[:, hc, oc * P:(oc + 1) * P],
                                     rhs[hc][:], start=(hc == 0),
                                     stop=(hc == 7))
                nc.scalar.activation(
                    out=qkvT[oc][:, blk * QB:(blk + 1) * QB], in_=ps[:],
                    func=AF.Identity, bias=qkvb_sb[:, oc:oc + 1], scale=1.0)

    # ============ Stage C+D: attention per (b, h) =========================
    with tc.tile_pool(name="vp_pool", bufs=NKT + 2) as vpp, \
         tc.tile_pool(name="e_pool", bufs=4) as ep, \
         tc.tile_pool(name="att_misc", bufs=4) as amp, \
         tc.tile_pool(name="s_ps", bufs=3, space="PSUM") as sps, \
         tc.tile_pool(name="ctx_ps", bufs=2, space="PSUM") as cps:
        for b in range(B):
            for h in range(HPC):
                hs = slice(h * HD, (h + 1) * HD)
                base = b * S
                vts = []
                for kt in range(NKT):
                    vp = vpp.tile([P, HD + 1], BF, tag="vp")
                    nc.sync.dma_start(
                        out=vp[:, 0:HD],
                        in_=qkvT[2][hs, base + kt * P: base + (kt + 1) * P],
                        transpose=True)
                    nc.vector.memset(vp[:, HD:HD + 1], 1.0)
                    vts.append(vp)
                for qb in range(4):
                    q0 = base + qb * QB
                    n_kt = 4 * (qb + 1)
                    cp = cps.tile([HD + 1, QB], F32, tag="ctxps")
                    for kt in range(n_kt):
                        sp = sps.tile([P, QB], F32, tag="sps")
                        nc.tensor.matmul(
                            sp[:],
                            qkvT[1][hs, base + kt * P: base + (kt + 1) * P],
                            qkvT[0][hs, q0:q0 + QB], start=True, stop=True)
                        e = ep.tile([P, QB], BF, tag="e")
                        nc.scalar.activation(out=e[:], in_=sp[:], func=AF.Exp,
                                             bias=0.0, scale=float(SCALE))
                        j = kt - 4 * qb
                        if j >= 0:
                            nc.vector.tensor_mul(e[:], e[:], masks_sb[:, j, :])
                        nc.tensor.matmul(cp[:], vts[kt][:], e[:],
                                         start=(kt == 0), stop=(kt == n_kt - 1))
                    recip = amp.tile([1, QB], F32, tag="recip")
                    nc.vector.reciprocal(out=recip[:], in_=cp[HD:HD + 1, :])
                    rb = amp.tile([HD, QB], F32, tag="rb")
                    nc.gpsimd.partition_broadcast(rb[:], recip[:])
                    cn = amp.tile([HD, QB], BF, tag="cn")
                    nc.vector.tensor_mul(cn[:], cp[0:HD, :], rb[:])
                    j_glob = 4 * b + qb
                    nc.sync.dma_start(
                        out=cc_c_in[j_glob * P + h * HD:
                                    j_glob * P + (h + 1) * HD, :],
                        in_=cn[:])

    nc.gpsimd.collective_compute(
        "AllToAll", mybir.AluOpType.bypass, replica_groups=RG,
        ins=[cc_c_in[:]], outs=[cc_c_out[:]])

    # ============ Stage E: attn_out + residual + LN2 ======================
    with tc.tile_pool(name="e_misc", bufs=3) as emp, \
         tc.tile_pool(name="e_ps", bufs=3, space="PSUM") as epp:
        for fc in range(NC):
            nc.sync.dma_start(out=ctxT_all[:, fc, :],
                              in_=cc_c_out[fc * P:(fc + 1) * P, :])
        for rt in range(4):
            rsl = slice(rt * P, (rt + 1) * P)
            for nh in range(2):
                ps = epp.tile([P, QB], F32, tag="eps")
                for fc in range(NC):
                    nc.tensor.matmul(ps[:], ctxT_all[:, fc, rsl],
                                     aow_sb[:, fc, nh * QB:(nh + 1) * QB],
                                     start=(fc == 0), stop=False)
                nc.tensor.matmul(ps[:], ones_sb[:],
                                 aob_sb[:, nh * QB:(nh + 1) * QB],
                                 start=False, stop=True)
                nc.vector.tensor_add(
                    resid_all[:, rt, nh * QB:(nh + 1) * QB], ps[:],
                    x_all[:, rt, nh * QB:(nh + 1) * QB])
            stats = emp.tile([P, 2, 6], F32, tag="stats2")
            mv = emp.tile([P, 2], F32, tag="mv2")
            nc.vector.bn_stats(out=stats[:, 0, :], in_=resid_all[:, rt, 0:512])
            nc.vector.bn_stats(out=stats[:, 1, :], in_=resid_all[:, rt, 512:H])
            nc.vector.bn_aggr(out=mv[:], in_=stats[:])
            std = emp.tile([P, 1], F32, tag="std2")
            nc.scalar.activation(out=std[:], in_=mv[:, 1:2], func=AF.Sqrt,
                                 bias=eps_sb[:], scale=1.0)
            rstd = emp.tile([P, 1], F32, tag="rstd2")
            nc.vector.reciprocal(out=rstd[:], in_=std[:])
            nmr = emp.tile([P, 1], F32, tag="nmr2")
            nc.vector.tensor_mul(nmr[:], mv[:, 0:1], rstd[:])
            nc.vector.tensor_scalar_mul(nmr[:], nmr[:], -1.0)
            t2 = emp.tile([P, H], BF, tag="t2")
            nc.scalar.activation(out=t2[:], in_=resid_all[:, rt, :],
                                 func=AF.Identity, bias=nmr[:], scale=rstd[:])
            for hc in range(8):
                nc.sync.dma_start(out=t2T_all[:, hc, rsl],
                                  in_=t2[:, hc * P:(hc + 1) * P],
                                  transpose=True)

    # ============ Stage F1: inter = gelu(t2T @ interw + b) ================
    interw_r = interw[:].rearrange("(c p) i -> c p i", p=P)
    with tc.tile_pool(name="iw_pool", bufs=10) as iwp, \
         tc.tile_pool(name="f1_ps", bufs=4, space="PSUM") as f1p:
        for icb in range(8):
            iws = []
            for hc in range(8):
                w = iwp.tile([P, 4 * P], BF, tag="iw")
                nc.sync.dma_start(
                    out=w[:],
                    in_=interw_r[hc, :, icb * 4 * P:(icb + 1) * 4 * P])
                iws.append(w)
            for i4 in range(4):
                ic = icb * 4 + i4
                ps = f1p.tile([P, RPC], F32, tag="f1ps")
                for hc in range(8):
                    nc.tensor.matmul(ps[:], iws[hc][:, i4 * P:(i4 + 1) * P],
                                     t2T_all[:, hc, :], start=(hc == 0),
                                     stop=(hc == 7))
                nc.scalar.activation(out=interT[:, ic, :], in_=ps[:],
                                     func=AF.Gelu_apprx_tanh,
                                     bias=interb_sb[:, ic:ic + 1], scale=1.0)

    # ============ Stage F2: out = interT.T @ outw + resid + b =============
    outw_r = outw[:].rearrange("(c p) h -> c p h", p=P)
    with tc.tile_pool(name="ow_pool", bufs=4) as owp, \
         tc.tile_pool(name="f2_ps", bufs=1, space="PSUM") as f2p, \
         tc.tile_pool(name="o_pool", bufs=3) as op:
        pss = [f2p.tile([P, H], F32, tag=f"f2ps{rt}", name=f"f2ps{rt}")
               for rt in range(4)]
        for ic in range(I // P):
            w = owp.tile([P, H], BF, tag="ow")
            nc.sync.dma_start(out=w[:], in_=outw_r[ic])
            for rt in range(4):
                for nh in range(2):
                    nc.tensor.matmul(
                        pss[rt][:, nh * QB:(nh + 1) * QB],
                        interT[:, ic, rt * P:(rt + 1) * P],
                        w[:, nh * QB:(nh + 1) * QB],
                        start=(ic == 0), stop=False)
        out_r = out[:].rearrange("(t p) h -> t p h", p=P)
        for rt in range(4):
            for nh in range(2):
                nc.tensor.matmul(pss[rt][:, nh * QB:(nh + 1) * QB],
                                 ones_sb[:], outb_sb[:, nh * QB:(nh + 1) * QB],
                                 start=False, stop=True)
            o = op.tile([P, H], F32, tag="o")
            nc.vector.tensor_add(o[:], pss[rt][:], resid_all[:, rt, :])
            nc.sync.dma_start(out=out_r[rt], in_=o[:])


def _get_nc():
    global _CACHED
    if _CACHED is None:
        _CACHED = _build()
    return _CACHED


def _prep_inputs(x, input_mask, norm_w, norm_b, qkvw, qkvb, attn_ow, attn_ob,
                 attn_nw, attn_nb, inter_w, inter_b, output_w, output_b):
    f64 = np.float64
    bf16 = ml_dtypes.bfloat16
    xf = np.ascontiguousarray(np.asarray(x), dtype=np.float32).reshape(NROWS, H)

    # fold LN1 into qkv weights, LN2 into inter weights (in fp64)
    qkvw_f = np.asarray(qkvw, f64) * np.asarray(norm_w, f64)[:, None]
    qkvb_f = np.asarray(qkvb, f64) + np.asarray(norm_b, f64) @ np.asarray(qkvw, f64)
    interw_f = np.asarray(inter_w, f64) * np.asarray(attn_nw, f64)[:, None]
    interb_f = np.asarray(inter_b, f64) + np.asarray(attn_nb, f64) @ np.asarray(inter_w, f64)

    interw_bf = np.ascontiguousarray(interw_f.astype(np.float32).astype(bf16))
    outw_bf = np.ascontiguousarray(np.asarray(output_w, np.float32).astype(bf16))
    aow_bf = np.ascontiguousarray(np.asarray(attn_ow, np.float32).astype(bf16))
    aob_bf = np.asarray(attn_ob, np.float32).astype(bf16).reshape(1, H)
    outb_bf = np.asarray(output_b, np.float32).astype(bf16).reshape(1, H)
    interb_dev = np.ascontiguousarray(
        interb_f.astype(np.float32).reshape(I // P, P).T)   # [128, 32]

    kk = np.arange(P)[:, None]
    qq = np.arange(QB)[None, :]
    masks = np.stack([(j * P + kk <= qq) for j in range(4)], axis=1)  # [P,4,QB]
    masks = np.ascontiguousarray(masks.astype(bf16))

    in_maps = []
    for c in range(NC):
        qs = np.concatenate([qkvw_f[:, P * c: P * (c + 1)],
                             qkvw_f[:, H + P * c: H + P * (c + 1)],
                             qkvw_f[:, 2 * H + P * c: 2 * H + P * (c + 1)]],
                            axis=1)                                   # [H, 384]
        qb_c = np.concatenate([qkvb_f[P * c: P * (c + 1)],
                               qkvb_f[H + P * c: H + P * (c + 1)],
                               qkvb_f[2 * H + P * c: 2 * H + P * (c + 1)]])
        qb_dev = np.ascontiguousarray(
            qb_c.astype(np.float32).reshape(3, P).T)                  # [128, 3]
        in_maps.append({
            "x_rows": np.ascontiguousarray(xf[RPC * c: RPC * (c + 1)]),
            "qkvw_s": np.ascontiguousarray(qs.astype(np.float32).astype(bf16)),
            "qkvb_s": qb_dev,
            "attn_ow": aow_bf,
            "attn_ob": aob_bf,
            "interw": interw_bf,
            "interb": interb_dev,
            "outw": outw_bf,
            "outb": outb_bf,
            "masks": masks,
        })
    return in_maps


def kernel(**inputs) -> np.ndarray:
    nc = _get_nc()
    in_maps = _prep_inputs(**inputs)
    res = run_bass_kernel_spmd(nc, in_maps, core_ids=list(range(NC)))
    last_run_info.clear()
    last_run_info.update(
        exec_time_ns=res.exec_time_ns,
        mean_exec_time_ns=res.mean_exec_time_ns,
        trace=res.instructions_and_trace[1] if res.instructions_and_trace else None,
    )
    outp = np.concatenate([r["out"] for r in res.results], axis=0)
    return outp.reshape(B, S, H)
